# revision 4
# baseline (speedup 1.0000x reference)
"""ContextualLoss on 8 Trainium2 NeuronCores (Bass/Tile).

Problem: nn_ContextualLoss — N=4, C=64, H=W=64, P=H*W=4096.

Math (per batch n):
  meanT    = mean of T over (N,H,W)                              [C]
  Tc/Ic    = centered features;  Tn = Tc/|Tc|, In = Ic/|Ic| per pixel (over C)
  cos[q,p] = In_q . Tn_p                                         [P, P]
  raw      = (1-cos)/2 ;  m_q = min_p raw = (1-max_p cos)/2
  cs_w     = exp(1 - raw/(m_q+eps)) = exp(sc_q*dot + bias_q)  where
             dot = Ic_q . Tn_p (g_q=1/|Ic_q| folded into the scale):
             a2_q = 1/(1+2eps - g_q*max_p dot),  sc_q = a2_q*g_q,  bias_q = 1-a2_q
  cs       = cs_w / sum_p cs_w ;  k_p = max_q cs[q,p]
  CS_n     = mean_p k_p ;  score = mean_n(-log CS_n)

Sharding: 2 cores per batch; each core owns 2048 q rows (all 4096 p columns),
so the row min/sum are core-local. Each core outputs its partial column-max
k [128, 4096] (partition i holds max over its 16 q-blocks); host reduces.
"""

import numpy as np

import concourse.bass as bass
import concourse.bacc as bacc_mod
import concourse.mybir as mybir
import concourse.tile as tile
from concourse.bass_utils import run_bass_kernel_spmd

N, C, H, W = 4, 64, 64, 64
P = H * W                  # 4096 template pixels
QH = P // 2                # 2048 query pixels per core
NBLK = QH // 128           # 16 q-blocks per core
NCORES = 8
EPS = 1e-5
F32 = mybir.dt.float32
BF16 = mybir.dt.bfloat16
AX = mybir.AxisListType
OP = mybir.AluOpType
AF = mybir.ActivationFunctionType

# dtype config (tuned after measuring error/perf)
MM_DT = F32        # main matmul input dtype: F32 | float32r | BF16
E_DT = F32         # ebuf / k dtype


def _mm_ap(ap):
    """View an AP in the configured matmul dtype."""
    if MM_DT == F32:
        return ap
    return ap.bitcast(MM_DT)


def build_nc():
    nc = bacc_mod.Bacc("TRN2", target_bir_lowering=False, debug=False)

    t_full = nc.dram_tensor("t_full", [128, 2 * P], F32, kind="ExternalInput")
    t_own = nc.dram_tensor("t_own", [C, P], F32, kind="ExternalInput")
    i_own = nc.dram_tensor("i_own", [C, QH], F32, kind="ExternalInput")
    k_out = nc.dram_tensor("k_out", [128, P], E_DT, kind="ExternalOutput")

    with tile.TileContext(nc) as tc:
        with (
            tc.tile_pool(name="persist", bufs=1) as pp,
            tc.tile_pool(name="small", bufs=4) as sp,
        ):
            # ---------------- persistent tiles ----------------
            ic = pp.tile([C, QH], F32)          # centered I slice (matmul lhsT)
            tn = pp.tile([C, P], F32)           # normalized T (matmul rhs)
            ktile = pp.tile([128, P], E_DT)     # running column max
            g2sb = pp.tile([128, NBLK], F32)    # |Ic_q|^2 in block layout
            g = pp.tile([128, NBLK], F32)       # 1/|Ic_q|
            negg = pp.tile([128, NBLK], F32)    # -g
            onecp = pp.tile([128, 1], F32)      # 1 + 2*eps
            ones64 = pp.tile([C, 1], F32)
            ones1 = pp.tile([1, C], F32)

            nc.vector.memset(ktile, 0.0)
            nc.vector.memset(onecp, 1.0 + 2.0 * EPS)
            nc.vector.memset(ones64, 1.0)
            nc.vector.memset(ones1, 1.0)

            # ---------------- prologue ----------------
            with (
                tc.tile_pool(name="pro", bufs=1) as pro,
                tc.tile_pool(name="pps", bufs=8, space="PSUM") as pps,
            ):
                # meanT: T rows packed [128, 8192]; partition p holds flat
                # rows p and 128+p of T[256, 4096]  (row r = n*64 + c)
                tf = pro.tile([128, 2 * P], F32)
                nc.sync.dma_start(out=tf, in_=t_full[:, :])
                tsc = pro.tile([128, 2 * P], BF16)  # mandatory activation out
                macc = sp.tile([128, 1], F32)
                nc.scalar.activation(out=tsc, in_=tf, func=AF.Copy,
                                     accum_out=macc)
                # meanT[c] = (macc[c] + macc[64+c]) / 16384; avoid multi-DMA
                # fan-in into one op (walrus sync-wait limit): scale, rotate
                # the upper half down via one DMA, subtract in two steps.
                ms = sp.tile([128, 1], F32)
                nc.vector.tensor_scalar_mul(ms, macc, 1.0 / (N * P))
                rot0 = sp.tile([C, 1], F32)
                nc.sync.dma_start(out=rot0, in_=ms[64:128, :])

                # center: x - ms[0:64] - rot0
                town = pro.tile([C, P], F32)
                iown = pro.tile([C, QH], F32)
                nc.sync.dma_start(out=town, in_=t_own[:, :])
                nc.sync.dma_start(out=iown, in_=i_own[:, :])
                tcent = pro.tile([C, P], F32)
                nc.vector.tensor_scalar(out=tcent, in0=town,
                                        scalar1=ms[0:C, 0:1], scalar2=None,
                                        op0=OP.subtract)
                nc.vector.tensor_scalar(out=tcent, in0=tcent, scalar1=rot0,
                                        scalar2=None, op0=OP.subtract)
                nc.vector.tensor_scalar(out=ic, in0=iown,
                                        scalar1=ms[0:C, 0:1], scalar2=None,
                                        op0=OP.subtract)
                nc.vector.tensor_scalar(out=ic, in0=ic, scalar1=rot0,
                                        scalar2=None, op0=OP.subtract)

                # squares
                sqt = pro.tile([C, P], F32)
                sqi = pro.tile([C, QH], F32)
                nc.scalar.square(sqt, tcent)
                nc.scalar.square(sqi, ic)

                # column sumsq of T via ones-matmul -> [1, 4096]
                normt = pro.tile([1, P], F32)
                npsums = []
                for c in range(8):
                    ps = pps.tile([1, 512], F32, tag="pps")
                    nc.tensor.matmul(ps, ones64, sqt[:, c * 512:(c + 1) * 512],
                                     start=True, stop=True)
                    npsums.append(ps)
                for c in range(8):
                    nc.scalar.activation(out=normt[:, c * 512:(c + 1) * 512],
                                         in_=npsums[c], func=AF.Sqrt)
                ht = pro.tile([1, P], F32)
                nc.vector.reciprocal(ht, normt)

                # broadcast h over C partitions (K=1 matmul) and form Tn
                for c in range(8):
                    ps = pps.tile([C, 512], F32, tag="pps")
                    nc.tensor.matmul(ps, ones1, ht[0:1, c * 512:(c + 1) * 512],
                                     start=True, stop=True)
                    nc.vector.tensor_tensor(
                        out=tn[:, c * 512:(c + 1) * 512],
                        in0=tcent[:, c * 512:(c + 1) * 512], in1=ps, op=OP.mult)

                # per-q sumsq of Ic in block layout [128, 16]
                for b in range(NBLK):
                    ps = pps.tile([128, 1], F32, tag="pps")
                    nc.tensor.matmul(ps, sqi[:, b * 128:(b + 1) * 128], ones64,
                                     start=True, stop=True)
                    nc.scalar.copy(g2sb[:, b:b + 1], ps)
                nc.vector.reciprocal(g, g2sb)
                nc.scalar.sqrt(g, g)                      # g = 1/|Ic_q|
                nc.vector.tensor_scalar_mul(negg, g, -1.0)

            # ---------------- main loop ----------------
            with (
                tc.tile_pool(name="ebuf", bufs=2) as ep,
                tc.tile_pool(name="mps", bufs=2, space="PSUM") as mps,
            ):
                for b in range(NBLK):
                    lhs = _mm_ap(ic[:, b * 128:(b + 1) * 128])
                    eb = ep.tile([128, P], E_DT, tag="eb")
                    rm2 = sp.tile([128, 2], F32, tag="rm2")
                    s2 = sp.tile([128, 2], F32, tag="s2")
                    pss = []
                    for h in range(2):
                        ps = mps.tile([128, 2048], F32, tag="mps")
                        for c in range(4):
                            nc.tensor.matmul(
                                ps[:, c * 512:(c + 1) * 512], lhs,
                                _mm_ap(tn[:, h * 2048 + c * 512:
                                          h * 2048 + (c + 1) * 512]),
                                start=True, stop=True)
                        nc.vector.reduce_max(out=rm2[:, h:h + 1], in_=ps,
                                             axis=AX.X)
                        pss.append(ps)
                    mq = sp.tile([128, 1], F32, tag="mq")
                    nc.vector.reduce_max(out=mq, in_=rm2, axis=AX.X)
                    dd = sp.tile([128, 1], F32, tag="dd")
                    nc.vector.scalar_tensor_tensor(
                        out=dd, in0=mq, scalar=negg[:, b:b + 1], in1=onecp,
                        op0=OP.mult, op1=OP.add)
                    a2 = sp.tile([128, 1], F32, tag="a2")
                    nc.vector.reciprocal(a2, dd)
                    sc = sp.tile([128, 1], F32, tag="sc")
                    nc.vector.tensor_tensor(out=sc, in0=a2, in1=g[:, b:b + 1],
                                            op=OP.mult)
                    bias = sp.tile([128, 1], F32, tag="bias")
                    nc.vector.tensor_scalar(out=bias, in0=a2, scalar1=-1.0,
                                            scalar2=1.0, op0=OP.mult,
                                            op1=OP.add)
                    for h in range(2):
                        nc.scalar.activation(
                            out=eb[:, h * 2048:(h + 1) * 2048], in_=pss[h],
                            func=AF.Exp, bias=bias, scale=sc,
                            accum_out=s2[:, h:h + 1])
                    ssum = sp.tile([128, 1], F32, tag="ssum")
                    nc.vector.tensor_tensor(out=ssum, in0=s2[:, 0:1],
                                            in1=s2[:, 1:2], op=OP.add)
                    rr = sp.tile([128, 1], F32, tag="rr")
                    nc.vector.reciprocal(rr, ssum)
                    # k = max(eb * r_q, k)
                    nc.vector.scalar_tensor_tensor(
                        out=ktile, in0=eb, scalar=rr, in1=ktile,
                        op0=OP.mult, op1=OP.max)

            nc.sync.dma_start(out=k_out[:, :], in_=ktile)

    nc.compile()
    return nc


_NC_CACHE = {}


def _get_nc():
    key = (str(MM_DT), str(E_DT))
    if key not in _NC_CACHE:
        _NC_CACHE[key] = build_nc()
    return _NC_CACHE[key]


def make_in_maps(I_features, T_features):
    I4 = np.ascontiguousarray(
        np.asarray(I_features, dtype=np.float32).reshape(N, C, P))
    T4 = np.ascontiguousarray(
        np.asarray(T_features, dtype=np.float32).reshape(N, C, P))
    # partition p holds flat T rows p and 128+p
    tf = np.ascontiguousarray(
        T4.reshape(2, 128, P).transpose(1, 0, 2).reshape(128, 2 * P))
    in_maps = []
    for core in range(NCORES):
        n, half = core // 2, core % 2
        in_maps.append({
            "t_full": tf,
            "t_own": np.ascontiguousarray(T4[n]),
            "i_own": np.ascontiguousarray(I4[n][:, half * QH:(half + 1) * QH]),
        })
    return in_maps


def finish_host(kparts):
    """kparts: [8, 128, P] per-core partial column maxima -> scalar score."""
    ks = np.stack([np.asarray(kp, dtype=np.float64) for kp in kparts])
    kp = ks.reshape(N, 2 * 128, P).max(axis=1)      # [N, P]
    cs = kp.mean(axis=1)                            # [N]
    return np.float32(np.mean(-np.log(cs)))


def kernel(I_features, T_features, _trace=False):
    nc = _get_nc()
    in_maps = make_in_maps(I_features, T_features)
    res = run_bass_kernel_spmd(nc, in_maps, core_ids=list(range(NCORES)),
                               trace=_trace)
    score = finish_host([r["k_out"] for r in res.results])
    if _trace:
        return np.array(score, dtype=np.float32), res
    return np.array(score, dtype=np.float32)


# revision 11
# speedup vs baseline: 1.2642x; 1.2642x over previous
"""ContextualLoss on 8 Trainium2 NeuronCores (Bass/Tile).

Problem: nn_ContextualLoss — N=4, C=64, H=W=64, P=H*W=4096.

Math (per batch n):
  meanT    = mean of T over (N,H,W)                              [C]
  Tc/Ic    = centered features;  Tn = Tc/|Tc|, In = Ic/|Ic| per pixel (over C)
  cos[q,p] = In_q . Tn_p                                         [P, P]
  raw      = (1-cos)/2 ;  m_q = min_p raw = (1-max_p cos)/2
  cs_w     = exp(1 - raw/(m_q+eps)) = exp(sc_q*dot + bias_q)  where
             dot = Ic_q . Tn_p (g_q=1/|Ic_q| folded into the scale):
             a2_q = 1/(1+2eps - g_q*max_p dot),  sc_q = a2_q*g_q,  bias_q = 1-a2_q
  cs       = cs_w / sum_p cs_w ;  k_p = max_q cs[q,p]
  CS_n     = mean_p k_p ;  score = mean_n(-log CS_n)

Sharding: 2 cores per batch; each core owns 2048 q rows (all 4096 p columns),
so the row min/sum are core-local. Each core outputs its partial column-max
k [128, 4096] (partition i holds max over its 16 q-blocks); host reduces.
"""

import numpy as np

import concourse.bass as bass
import concourse.bacc as bacc_mod
import concourse.mybir as mybir
import concourse.tile as tile
from concourse.bass_utils import run_bass_kernel_spmd

N, C, H, W = 4, 64, 64, 64
P = H * W                  # 4096 template pixels
QH = P // 2                # 2048 query pixels per core
NBLK = QH // 128           # 16 q-blocks per core
NCORES = 8
EPS = 1e-5
F32 = mybir.dt.float32
BF16 = mybir.dt.bfloat16
AX = mybir.AxisListType
OP = mybir.AluOpType
AF = mybir.ActivationFunctionType

# dtype config (tuned after measuring error/perf)
F32R = mybir.dt.float32r
MM_DT = F32R       # main matmul input dtype: F32 | float32r | BF16
E_DT = BF16        # ebuf / k dtype


# Tiles consumed by matmuls are allocated in MM_DT directly: the BIR
# verifier requires fp32r matmul operands to be *produced* as fp32r.


def build_nc():
    nc = bacc_mod.Bacc("TRN2", target_bir_lowering=False, debug=False)

    t_full = nc.dram_tensor("t_full", [128, 2 * P], F32, kind="ExternalInput")
    t_own = nc.dram_tensor("t_own", [C, P], F32, kind="ExternalInput")
    i_own = nc.dram_tensor("i_own", [C, QH], F32, kind="ExternalInput")
    k_out = nc.dram_tensor("k_out", [128, P], E_DT, kind="ExternalOutput")

    with tile.TileContext(nc) as tc:
        with (
            tc.tile_pool(name="persist", bufs=1) as pp,
            tc.tile_pool(name="small", bufs=4) as sp,
        ):
            # ---------------- persistent tiles ----------------
            ic = pp.tile([C, QH], MM_DT)        # centered I slice (matmul lhsT)
            tn = pp.tile([C, P], MM_DT)         # normalized T (matmul rhs)
            ktile = pp.tile([128, P], E_DT)     # running column max
            g2sb = pp.tile([128, NBLK], F32)    # |Ic_q|^2 in block layout
            g = pp.tile([128, NBLK], F32)       # 1/|Ic_q|
            negg = pp.tile([128, NBLK], F32)    # -g
            onecp = pp.tile([128, 1], F32)      # 1 + 2*eps
            ones64 = pp.tile([C, 1], MM_DT)
            ones1 = pp.tile([1, C], MM_DT)

            nc.vector.memset(ktile, 0.0)
            nc.vector.memset(onecp, 1.0 + 2.0 * EPS)
            # memset can't produce fp32r; stage in f32 and copy through ACT
            ones64f = pp.tile([C, 1], F32)
            ones1f = pp.tile([1, C], F32)
            nc.vector.memset(ones64f, 1.0)
            nc.vector.memset(ones1f, 1.0)
            nc.scalar.copy(ones64, ones64f)
            nc.scalar.copy(ones1, ones1f)

            # ---------------- prologue ----------------
            with (
                tc.tile_pool(name="pro", bufs=1) as pro,
                tc.tile_pool(name="pps", bufs=8, space="PSUM") as pps,
            ):
                # meanT: T rows packed [128, 8192]; partition p holds flat
                # rows p and 128+p of T[256, 4096]  (row r = n*64 + c)
                tf = pro.tile([128, 2 * P], F32)
                nc.sync.dma_start(out=tf, in_=t_full[:, :])
                tsc = pro.tile([128, 2 * P], BF16)  # mandatory activation out
                macc = sp.tile([128, 1], F32)
                nc.scalar.activation(out=tsc, in_=tf, func=AF.Copy,
                                     accum_out=macc)
                # meanT[c] = (macc[c] + macc[64+c]) / 16384; avoid multi-DMA
                # fan-in into one op (walrus sync-wait limit): scale, rotate
                # the upper half down via one DMA, subtract in two steps.
                ms = sp.tile([128, 1], F32)
                nc.vector.tensor_scalar_mul(ms, macc, 1.0 / (N * P))
                rot0 = sp.tile([C, 1], F32)
                nc.sync.dma_start(out=rot0, in_=ms[64:128, :])

                # center: x - ms[0:64] - rot0
                town = pro.tile([C, P], F32)
                iown = pro.tile([C, QH], F32)
                nc.sync.dma_start(out=town, in_=t_own[:, :])
                nc.sync.dma_start(out=iown, in_=i_own[:, :])
                tcent = pro.tile([C, P], F32)
                nc.vector.tensor_scalar(out=tcent, in0=town,
                                        scalar1=ms[0:C, 0:1], scalar2=None,
                                        op0=OP.subtract)
                nc.vector.tensor_scalar(out=tcent, in0=tcent, scalar1=rot0,
                                        scalar2=None, op0=OP.subtract)
                nc.vector.tensor_scalar(out=ic, in0=iown,
                                        scalar1=ms[0:C, 0:1], scalar2=None,
                                        op0=OP.subtract)
                nc.vector.tensor_scalar(out=ic, in0=ic, scalar1=rot0,
                                        scalar2=None, op0=OP.subtract)

                # squares
                sqt = pro.tile([C, P], MM_DT)
                # fp32r is invalid ISA for moving-free-dim N=1; the per-q
                # sumsq matmuls stay plain fp32
                sqi = pro.tile([C, QH], F32)
                nc.scalar.square(sqt, tcent)
                nc.scalar.square(sqi, ic)

                # column sumsq of T via ones-matmul -> [1, 4096], then
                # h = ssq^-1/2 = exp(-0.5*ln(ssq)) on ACT (DVE reciprocal is
                # ~6 cyc/elem on the free dim -> 25us for [1,4096]; avoid)
                lnb = pro.tile([1, P], F32)
                npsums = []
                for c in range(8):
                    ps = pps.tile([1, 512], F32, tag="pps")
                    nc.tensor.matmul(ps, ones64,
                                     sqt[:, c * 512:(c + 1) * 512],
                                     start=True, stop=True)
                    npsums.append(ps)
                for c in range(8):
                    nc.scalar.activation(out=lnb[:, c * 512:(c + 1) * 512],
                                         in_=npsums[c], func=AF.Ln)
                ht = pro.tile([1, P], MM_DT)
                nc.scalar.activation(out=ht, in_=lnb, func=AF.Exp, scale=-0.5)

                # broadcast h over C partitions (K=1 matmul) and form Tn
                for c in range(8):
                    ps = pps.tile([C, 512], F32, tag="pps")
                    nc.tensor.matmul(ps, ones1,
                                     ht[0:1, c * 512:(c + 1) * 512],
                                     start=True, stop=True)
                    nc.vector.tensor_tensor(
                        out=tn[:, c * 512:(c + 1) * 512],
                        in0=tcent[:, c * 512:(c + 1) * 512], in1=ps, op=OP.mult)

                # per-q sumsq of Ic in block layout [128, 16]
                for b in range(NBLK):
                    ps = pps.tile([128, 1], F32, tag="pps")
                    nc.tensor.matmul(ps, sqi[:, b * 128:(b + 1) * 128],
                                     ones64f, start=True, stop=True)
                    nc.scalar.copy(g2sb[:, b:b + 1], ps)
                nc.vector.reciprocal(g, g2sb)
                nc.scalar.sqrt(g, g)                      # g = 1/|Ic_q|
                nc.vector.tensor_scalar_mul(negg, g, -1.0)

            # ---------------- main loop ----------------
            with (
                tc.tile_pool(name="ebuf", bufs=2) as ep,
                tc.tile_pool(name="mps", bufs=2, space="PSUM") as mps,
            ):
                for b in range(NBLK):
                    lhs = ic[:, b * 128:(b + 1) * 128]
                    eb = ep.tile([128, P], E_DT, tag="eb")
                    rm2 = sp.tile([128, 2], F32, tag="rm2")
                    s2 = sp.tile([128, 2], F32, tag="s2")
                    pss = []
                    for h in range(2):
                        ps = mps.tile([128, 2048], F32, tag="mps")
                        for c in range(4):
                            nc.tensor.matmul(
                                ps[:, c * 512:(c + 1) * 512], lhs,
                                tn[:, h * 2048 + c * 512:
                                   h * 2048 + (c + 1) * 512],
                                start=True, stop=True)
                        nc.vector.reduce_max(out=rm2[:, h:h + 1], in_=ps,
                                             axis=AX.X)
                        pss.append(ps)
                    mq = sp.tile([128, 1], F32, tag="mq")
                    nc.vector.reduce_max(out=mq, in_=rm2, axis=AX.X)
                    dd = sp.tile([128, 1], F32, tag="dd")
                    nc.vector.scalar_tensor_tensor(
                        out=dd, in0=mq, scalar=negg[:, b:b + 1], in1=onecp,
                        op0=OP.mult, op1=OP.add)
                    a2 = sp.tile([128, 1], F32, tag="a2")
                    nc.vector.reciprocal(a2, dd)
                    sc = sp.tile([128, 1], F32, tag="sc")
                    nc.vector.tensor_tensor(out=sc, in0=a2, in1=g[:, b:b + 1],
                                            op=OP.mult)
                    bias = sp.tile([128, 1], F32, tag="bias")
                    nc.vector.tensor_scalar(out=bias, in0=a2, scalar1=-1.0,
                                            scalar2=1.0, op0=OP.mult,
                                            op1=OP.add)
                    for h in range(2):
                        nc.scalar.activation(
                            out=eb[:, h * 2048:(h + 1) * 2048], in_=pss[h],
                            func=AF.Exp, bias=bias, scale=sc,
                            accum_out=s2[:, h:h + 1])
                    ssum = sp.tile([128, 1], F32, tag="ssum")
                    nc.vector.tensor_tensor(out=ssum, in0=s2[:, 0:1],
                                            in1=s2[:, 1:2], op=OP.add)
                    rr = sp.tile([128, 1], F32, tag="rr")
                    nc.vector.reciprocal(rr, ssum)
                    # k = max(eb * r_q, k); ts (4x bf16) + tt (2x bf16) beats
                    # the fused scalar_tensor_tensor (1x only)
                    cs = ep.tile([128, P], E_DT, tag="cs")
                    nc.vector.tensor_scalar(out=cs, in0=eb, scalar1=rr,
                                            scalar2=None, op0=OP.mult)
                    nc.vector.tensor_tensor(out=ktile, in0=ktile, in1=cs,
                                            op=OP.max)

            nc.sync.dma_start(out=k_out[:, :], in_=ktile)

    nc.compile()
    return nc


_NC_CACHE = {}


def _get_nc():
    key = (str(MM_DT), str(E_DT))
    if key not in _NC_CACHE:
        _NC_CACHE[key] = build_nc()
    return _NC_CACHE[key]


def make_in_maps(I_features, T_features):
    I4 = np.ascontiguousarray(
        np.asarray(I_features, dtype=np.float32).reshape(N, C, P))
    T4 = np.ascontiguousarray(
        np.asarray(T_features, dtype=np.float32).reshape(N, C, P))
    # partition p holds flat T rows p and 128+p
    tf = np.ascontiguousarray(
        T4.reshape(2, 128, P).transpose(1, 0, 2).reshape(128, 2 * P))
    in_maps = []
    for core in range(NCORES):
        n, half = core // 2, core % 2
        in_maps.append({
            "t_full": tf,
            "t_own": np.ascontiguousarray(T4[n]),
            "i_own": np.ascontiguousarray(I4[n][:, half * QH:(half + 1) * QH]),
        })
    return in_maps


def finish_host(kparts):
    """kparts: [8, 128, P] per-core partial column maxima -> scalar score."""
    ks = np.stack([np.asarray(kp, dtype=np.float64) for kp in kparts])
    kp = ks.reshape(N, 2 * 128, P).max(axis=1)      # [N, P]
    cs = kp.mean(axis=1)                            # [N]
    return np.float32(np.mean(-np.log(cs)))


def kernel(I_features, T_features, _trace=False):
    nc = _get_nc()
    in_maps = make_in_maps(I_features, T_features)
    res = run_bass_kernel_spmd(nc, in_maps, core_ids=list(range(NCORES)),
                               trace=_trace)
    score = finish_host([r["k_out"] for r in res.results])
    if _trace:
        return np.array(score, dtype=np.float32), res
    return np.array(score, dtype=np.float32)


# revision 12
# speedup vs baseline: 1.5998x; 1.2654x over previous
"""ContextualLoss on 8 Trainium2 NeuronCores (Bass/Tile).

Problem: nn_ContextualLoss — N=4, C=64, H=W=64, P=H*W=4096.

Math (per batch n):
  meanT    = mean of T over (N,H,W)                              [C]
  Tc/Ic    = centered features;  Tn = Tc/|Tc|, In = Ic/|Ic| per pixel (over C)
  cos[q,p] = In_q . Tn_p                                         [P, P]
  raw      = (1-cos)/2 ;  m_q = min_p raw = (1-max_p cos)/2
  cs_w     = exp(1 - raw/(m_q+eps)) = exp(sc_q*dot + bias_q)  where
             dot = Ic_q . Tn_p (g_q=1/|Ic_q| folded into the scale):
             a2_q = 1/(1+2eps - g_q*max_p dot),  sc_q = a2_q*g_q,  bias_q = 1-a2_q
  cs       = cs_w / sum_p cs_w ;  k_p = max_q cs[q,p]
  CS_n     = mean_p k_p ;  score = mean_n(-log CS_n)

Sharding: 2 cores per batch; each core owns 2048 q rows (all 4096 p columns),
so the row min/sum are core-local. Each core outputs its partial column-max
k [128, 4096] (partition i holds max over its 16 q-blocks); host reduces.
"""

import numpy as np

import concourse.bass as bass
import concourse.bacc as bacc_mod
import concourse.mybir as mybir
import concourse.tile as tile
from concourse.bass_utils import run_bass_kernel_spmd

N, C, H, W = 4, 64, 64, 64
P = H * W                  # 4096 template pixels
QH = P // 2                # 2048 query pixels per core
NBLK = QH // 128           # 16 q-blocks per core
NCORES = 8
EPS = 1e-5
F32 = mybir.dt.float32
BF16 = mybir.dt.bfloat16
AX = mybir.AxisListType
OP = mybir.AluOpType
AF = mybir.ActivationFunctionType

# dtype config (tuned after measuring error/perf)
F32R = mybir.dt.float32r
MM_DT = F32R       # main matmul input dtype: F32 | float32r | BF16
E_DT = BF16        # ebuf / k dtype


# Tiles consumed by matmuls are allocated in MM_DT directly: the BIR
# verifier requires fp32r matmul operands to be *produced* as fp32r.


def build_nc():
    nc = bacc_mod.Bacc("TRN2", target_bir_lowering=False, debug=False)

    t_full = nc.dram_tensor("t_full", [128, 2 * P], F32, kind="ExternalInput")
    t_own = nc.dram_tensor("t_own", [C, P], F32, kind="ExternalInput")
    i_own = nc.dram_tensor("i_own", [C, QH], F32, kind="ExternalInput")
    k_out = nc.dram_tensor("k_out", [128, P], E_DT, kind="ExternalOutput")

    with tile.TileContext(nc) as tc:
        with (
            tc.tile_pool(name="persist", bufs=1) as pp,
            tc.tile_pool(name="small", bufs=4) as sp,
        ):
            # ---------------- persistent tiles ----------------
            ic = pp.tile([C, QH], MM_DT)        # centered I slice (matmul lhsT)
            tn = pp.tile([C, P], MM_DT)         # normalized T (matmul rhs)
            ktile = pp.tile([128, P], E_DT)     # running column max
            g2sb = pp.tile([128, NBLK], F32)    # |Ic_q|^2 in block layout
            g = pp.tile([128, NBLK], F32)       # 1/|Ic_q|
            negg = pp.tile([128, NBLK], F32)    # -g
            onecp = pp.tile([128, 1], F32)      # 1 + 2*eps
            ones64 = pp.tile([C, 1], MM_DT)
            ones1 = pp.tile([1, C], MM_DT)

            nc.vector.memset(ktile, 0.0)
            nc.vector.memset(onecp, 1.0 + 2.0 * EPS)
            # memset can't produce fp32r; stage in f32 and copy through ACT
            ones64f = pp.tile([C, 1], F32)
            ones1f = pp.tile([1, C], F32)
            nc.vector.memset(ones64f, 1.0)
            nc.vector.memset(ones1f, 1.0)
            nc.scalar.copy(ones64, ones64f)
            nc.scalar.copy(ones1, ones1f)

            # ---------------- prologue ----------------
            with (
                tc.tile_pool(name="pro", bufs=1) as pro,
                tc.tile_pool(name="pps", bufs=8, space="PSUM") as pps,
            ):
                # meanT: T rows packed [128, 8192]; partition p holds flat
                # rows p and 128+p of T[256, 4096]  (row r = n*64 + c)
                tf = pro.tile([128, 2 * P], F32)
                nc.sync.dma_start(out=tf, in_=t_full[:, :])
                tsc = pro.tile([128, 2 * P], BF16)  # mandatory activation out
                macc = sp.tile([128, 1], F32)
                nc.scalar.activation(out=tsc, in_=tf, func=AF.Copy,
                                     accum_out=macc)
                # meanT[c] = (macc[c] + macc[64+c]) / 16384; avoid multi-DMA
                # fan-in into one op (walrus sync-wait limit): scale, rotate
                # the upper half down via one DMA, subtract in two steps.
                ms = sp.tile([128, 1], F32)
                nc.vector.tensor_scalar_mul(ms, macc, 1.0 / (N * P))
                rot0 = sp.tile([C, 1], F32)
                nc.sync.dma_start(out=rot0, in_=ms[64:128, :])

                # center: x - ms[0:64] - rot0
                town = pro.tile([C, P], F32)
                iown = pro.tile([C, QH], F32)
                nc.sync.dma_start(out=town, in_=t_own[:, :])
                nc.sync.dma_start(out=iown, in_=i_own[:, :])
                tcent = pro.tile([C, P], F32)
                nc.vector.tensor_scalar(out=tcent, in0=town,
                                        scalar1=ms[0:C, 0:1], scalar2=None,
                                        op0=OP.subtract)
                nc.vector.tensor_scalar(out=tcent, in0=tcent, scalar1=rot0,
                                        scalar2=None, op0=OP.subtract)
                nc.vector.tensor_scalar(out=ic, in0=iown,
                                        scalar1=ms[0:C, 0:1], scalar2=None,
                                        op0=OP.subtract)
                nc.vector.tensor_scalar(out=ic, in0=ic, scalar1=rot0,
                                        scalar2=None, op0=OP.subtract)

                # squares
                sqt = pro.tile([C, P], MM_DT)
                # fp32r is invalid ISA for moving-free-dim N=1; the per-q
                # sumsq matmuls stay plain fp32
                sqi = pro.tile([C, QH], F32)
                nc.scalar.square(sqt, tcent)
                nc.scalar.square(sqi, ic)

                # column sumsq of T via ones-matmul -> [1, 4096], then
                # h = ssq^-1/2 = exp(-0.5*ln(ssq)) on ACT (DVE reciprocal is
                # ~6 cyc/elem on the free dim -> 25us for [1,4096]; avoid)
                lnb = pro.tile([1, P], F32)
                npsums = []
                for c in range(8):
                    ps = pps.tile([1, 512], F32, tag="pps")
                    nc.tensor.matmul(ps, ones64,
                                     sqt[:, c * 512:(c + 1) * 512],
                                     start=True, stop=True)
                    npsums.append(ps)
                for c in range(8):
                    nc.scalar.activation(out=lnb[:, c * 512:(c + 1) * 512],
                                         in_=npsums[c], func=AF.Ln)
                ht = pro.tile([1, P], MM_DT)
                nc.scalar.activation(out=ht, in_=lnb, func=AF.Exp, scale=-0.5)

                # broadcast h over C partitions (K=1 matmul) and form Tn
                for c in range(8):
                    ps = pps.tile([C, 512], F32, tag="pps")
                    nc.tensor.matmul(ps, ones1,
                                     ht[0:1, c * 512:(c + 1) * 512],
                                     start=True, stop=True)
                    nc.vector.tensor_tensor(
                        out=tn[:, c * 512:(c + 1) * 512],
                        in0=tcent[:, c * 512:(c + 1) * 512], in1=ps, op=OP.mult)

                # per-q sumsq of Ic in block layout [128, 16]
                for b in range(NBLK):
                    ps = pps.tile([128, 1], F32, tag="pps")
                    nc.tensor.matmul(ps, sqi[:, b * 128:(b + 1) * 128],
                                     ones64f, start=True, stop=True)
                    nc.scalar.copy(g2sb[:, b:b + 1], ps)
                nc.vector.reciprocal(g, g2sb)
                nc.scalar.sqrt(g, g)                      # g = 1/|Ic_q|
                nc.vector.tensor_scalar_mul(negg, g, -1.0)

            # ---------------- main loop ----------------
            # PSUM is tiled [128,1024] x 4 (2 banks each) so banks release
            # incrementally after their exp pass and the next block's
            # matmuls pipeline in behind this block's tail.
            NPS = 4
            PW = P // NPS
            with (
                tc.tile_pool(name="ebuf", bufs=2) as ep,
                tc.tile_pool(name="mps", bufs=NPS, space="PSUM") as mps,
            ):
                for b in range(NBLK):
                    lhs = ic[:, b * 128:(b + 1) * 128]
                    eb = ep.tile([128, P], E_DT, tag="eb")
                    rm4 = sp.tile([128, NPS], F32, tag="rm4")
                    s4 = sp.tile([128, NPS], F32, tag="s4")
                    pss = []
                    for h in range(NPS):
                        ps = mps.tile([128, PW], F32, tag="mps")
                        for c in range(PW // 512):
                            nc.tensor.matmul(
                                ps[:, c * 512:(c + 1) * 512], lhs,
                                tn[:, h * PW + c * 512:
                                   h * PW + (c + 1) * 512],
                                start=True, stop=True)
                        nc.vector.reduce_max(out=rm4[:, h:h + 1], in_=ps,
                                             axis=AX.X)
                        pss.append(ps)
                    mq = sp.tile([128, 1], F32, tag="mq")
                    nc.vector.reduce_max(out=mq, in_=rm4, axis=AX.X)
                    dd = sp.tile([128, 1], F32, tag="dd")
                    nc.vector.scalar_tensor_tensor(
                        out=dd, in0=mq, scalar=negg[:, b:b + 1], in1=onecp,
                        op0=OP.mult, op1=OP.add)
                    a2 = sp.tile([128, 1], F32, tag="a2")
                    nc.vector.reciprocal(a2, dd)
                    sc = sp.tile([128, 1], F32, tag="sc")
                    nc.vector.tensor_tensor(out=sc, in0=a2, in1=g[:, b:b + 1],
                                            op=OP.mult)
                    bias = sp.tile([128, 1], F32, tag="bias")
                    nc.vector.tensor_scalar(out=bias, in0=a2, scalar1=-1.0,
                                            scalar2=1.0, op0=OP.mult,
                                            op1=OP.add)
                    for h in range(NPS):
                        nc.scalar.activation(
                            out=eb[:, h * PW:(h + 1) * PW], in_=pss[h],
                            func=AF.Exp, bias=bias, scale=sc,
                            accum_out=s4[:, h:h + 1])
                    ssum = sp.tile([128, 1], F32, tag="ssum")
                    nc.vector.reduce_sum(out=ssum, in_=s4, axis=AX.X)
                    rr = sp.tile([128, 1], F32, tag="rr")
                    nc.vector.reciprocal(rr, ssum)
                    # k = max(eb * r_q, k); ts (4x bf16) + tt (2x bf16) beats
                    # the fused scalar_tensor_tensor (1x only)
                    cs = ep.tile([128, P], E_DT, tag="cs")
                    nc.vector.tensor_scalar(out=cs, in0=eb, scalar1=rr,
                                            scalar2=None, op0=OP.mult)
                    nc.vector.tensor_tensor(out=ktile, in0=ktile, in1=cs,
                                            op=OP.max)

            nc.sync.dma_start(out=k_out[:, :], in_=ktile)

    nc.compile()
    return nc


_NC_CACHE = {}


def _get_nc():
    key = (str(MM_DT), str(E_DT))
    if key not in _NC_CACHE:
        _NC_CACHE[key] = build_nc()
    return _NC_CACHE[key]


def make_in_maps(I_features, T_features):
    I4 = np.ascontiguousarray(
        np.asarray(I_features, dtype=np.float32).reshape(N, C, P))
    T4 = np.ascontiguousarray(
        np.asarray(T_features, dtype=np.float32).reshape(N, C, P))
    # partition p holds flat T rows p and 128+p
    tf = np.ascontiguousarray(
        T4.reshape(2, 128, P).transpose(1, 0, 2).reshape(128, 2 * P))
    in_maps = []
    for core in range(NCORES):
        n, half = core // 2, core % 2
        in_maps.append({
            "t_full": tf,
            "t_own": np.ascontiguousarray(T4[n]),
            "i_own": np.ascontiguousarray(I4[n][:, half * QH:(half + 1) * QH]),
        })
    return in_maps


def finish_host(kparts):
    """kparts: [8, 128, P] per-core partial column maxima -> scalar score."""
    ks = np.stack([np.asarray(kp, dtype=np.float64) for kp in kparts])
    kp = ks.reshape(N, 2 * 128, P).max(axis=1)      # [N, P]
    cs = kp.mean(axis=1)                            # [N]
    return np.float32(np.mean(-np.log(cs)))


def kernel(I_features, T_features, _trace=False):
    nc = _get_nc()
    in_maps = make_in_maps(I_features, T_features)
    res = run_bass_kernel_spmd(nc, in_maps, core_ids=list(range(NCORES)),
                               trace=_trace)
    score = finish_host([r["k_out"] for r in res.results])
    if _trace:
        return np.array(score, dtype=np.float32), res
    return np.array(score, dtype=np.float32)
